# revision 21
# baseline (speedup 1.0000x reference)
"""Multi-head self-attention with RoPE — Trainium2 Bass kernel, 8 NeuronCores.

Sharding: core c = 2*b + g handles batch b = c//2 and head-group g = c%2
(8 of the 16 heads).  Per-pair AllGather of normalized attention outputs
(O^T, bf16); each core runs the output projection for HALF of w_out's
columns (host concatenates the two halves).

Per-core dataflow (matmuls bf16, fp32 PSUM accumulation):
  xT [E, L] bf16 (pre-transposed on host)
  QKV:   Q^T/K^T pair tiles via W-stationary matmuls; V natural [L, 512].
  RoPE:  weights pre-permuted on host to de-interleave even/odd dims;
         rotate-half fused into 32-partition cross-quadrant DVE muls.
  Scores:S^T half-tiles [Lk=128, Lq=512] per head; the two heads of a
         pair sit on PE row-groups 0-63 / 64-127 and run concurrently.
  exp:   groups of 2 chunks [128, 1024 psum]; a subset of chunks runs on
         DVE as a 1-instruction Schraudolph exp (fp32 -> int16 bitcast
         bf16), the rest on ACT.  Software-pipelined so PE never waits.
  AV:    O^T[65, 512] += V_aug^T A^T over 16 Lk chunks (ones column
         yields the softmax denominator in row 64).
  Norm:  reciprocal + gpsimd partition_broadcast; the per-head multiply
         writes straight into the stacked [128, chunk, L] layout.
  Proj:  y[:, half] = Ocat^T.T @ w_out[:, half], fp32 [L, E/2].
"""

import contextlib
import functools

import numpy as np
import ml_dtypes

import concourse.bass as bass
import concourse.mybir as mybir
import concourse.tile as tile
from concourse import bacc
from concourse.bass_utils import run_bass_kernel_spmd

BF16 = mybir.dt.bfloat16
F32 = mybir.dt.float32
I16 = mybir.dt.int16
N_CORES = 8
ROPE_THETA = 10000.0

B_FULL, L_FULL, E_FULL = 4, 2048, 1024
H_FULL = 16

# lk chunk indices (0..LKC-1) whose exp runs on DVE (Schraudolph int16
# bit trick) instead of ACT.  Spread evenly; size tunes the engine split.
DVE_EXP_LKS = frozenset({2, 5, 8, 11, 14})
# Schraudolph constants for bf16 target: int16 = round(s*SCH_A + SCH_B),
# bitcast bf16 ~= exp(s * scale).  SCH_C tunes the max-error centering.
SCH_C = -5.5


def _emit(tc, nc, xT, wqkv, wout, cosT, sinT, y, L, E, HC, D, taps=None, use_collective=True):
    P = 128
    EC = E // P                 # E chunks of 128 (contraction)
    NPAIR = HC // 2             # head pairs per core
    LT = L // 512               # 512-wide L tiles
    LKC = L // P                # 128-wide Lk chunks
    A = HC * D                  # local attention width (512)
    EH = E // 2                 # output columns per core
    scale = 1.0 / float(np.sqrt(D))
    Exp = mybir.ActivationFunctionType.Exp
    sch_a = scale * 128.0 / float(np.log(2.0))
    sch_b = 127.0 * 128.0 + SCH_C

    ctx = contextlib.ExitStack()
    pool = ctx.enter_context(tc.tile_pool(name="sb", bufs=1))
    psum = ctx.enter_context(tc.tile_pool(name="ps", bufs=1, space="PSUM"))
    work = ctx.enter_context(tc.tile_pool(name="wk", bufs=1))
    dram = ctx.enter_context(tc.tile_pool(name="dr", bufs=1, space="DRAM"))

    # ---- persistent SBUF buffers ----
    xt_sb = pool.tile([P, EC, L], BF16, tag="xbuf")
    wqkv_sb = pool.tile([P, EC, 3 * A], BF16, tag="wqkv")
    wout_sb = pool.tile([P, EC, EH], BF16, tag="wout")
    cos_sb = pool.tile([P, LT, 1024], BF16, tag="costab")   # q|k duplicated
    sin_sb = pool.tile([P, LT, 1024], BF16, tag="sintab")
    qk_sb = pool.tile([P, NPAIR, LT, 1024], BF16, tag="qk")  # [rows, pair, ltile, q|k]
    vaug_sb = pool.tile([P, LKC, HC, D + 1], BF16, tag="vaug")

    for e in range(EC):
        nc.sync.dma_start(xt_sb[:, e, :], xT.ap()[e * P : (e + 1) * P, :])
        nc.sync.dma_start(wqkv_sb[:, e, :], wqkv.ap()[e * P : (e + 1) * P, :])
    nc.sync.dma_start(wout_sb[:], wout.ap().rearrange("(c p) n -> p c n", p=P))
    nc.sync.dma_start(cos_sb[:], cosT.ap())
    nc.sync.dma_start(sin_sb[:], sinT.ap())

    # ones column for the softmax denominator
    nc.vector.memset(vaug_sb[:, :, :, D : D + 1], 1.0)

    # ---- V = x @ Wv, natural [L, A] layout, 2 L-chunks per PSUM tile ----
    for vg in range(LKC // 2):
        ps = psum.tile([P, 1024], F32, tag="sc", bufs=3)
        for i in range(2):
            lt = vg * 2 + i
            for e in range(EC):
                nc.tensor.matmul(
                    ps[:, i * 512 : (i + 1) * 512],
                    lhsT=xt_sb[:, e, lt * P : (lt + 1) * P],
                    rhs=wqkv_sb[:, e, 2 * A : 3 * A],
                    start=(e == 0),
                    stop=(e == EC - 1),
                )
        nc.scalar.copy(
            out=vaug_sb[:, vg * 2 : (vg + 1) * 2, :, 0:D],
            in_=ps[:].rearrange("p (t h d) -> p t h d", h=HC, d=D),
        )

    # ---- Q^T / K^T + RoPE (muls/adds on DVE, swap fused cross-quadrant) ----
    for p in range(NPAIR):
        for lt in range(LT):
            ps = psum.tile([P, 1024], F32, tag="sc", bufs=3)
            for qk in range(2):
                wcol = qk * A + p * P
                for e in range(EC):
                    nc.tensor.matmul(
                        ps[:, qk * 512 : (qk + 1) * 512],
                        lhsT=wqkv_sb[:, e, wcol : wcol + P],
                        rhs=xt_sb[:, e, lt * 512 : (lt + 1) * 512],
                        start=(e == 0),
                        stop=(e == EC - 1),
                    )
            qs = work.tile([P, 1024], BF16, tag="qs", bufs=4)
            nc.scalar.copy(out=qs[:], in_=ps[:])
            t = work.tile([P, 1024], BF16, tag="ropet", bufs=3)
            w = work.tile([P, 1024], BF16, tag="ropew", bufs=3)
            nc.gpsimd.tensor_mul(t[:], qs[:], cos_sb[:, lt, :])
            # w[blk] = qs[blk^1] * sin[blk^1]  (rotate-half, sin pre-signed)
            for blk in range(4):
                sb = blk ^ 1
                nc.vector.tensor_mul(
                    w[blk * 32 : (blk + 1) * 32, :],
                    qs[sb * 32 : (sb + 1) * 32, :],
                    sin_sb[sb * 32 : (sb + 1) * 32, lt, :],
                )
            nc.gpsimd.tensor_add(qk_sb[:, p, lt, :], t[:], w[:])

    # ---- attention, software-pipelined over lk chunks ----
    # ocat reuses xt's buffer (xt is dead once Q/K/V are built).  Own
    # normalized pairs land in chunks 0..3 (local order); the AllGather
    # unpack then rewrites chunks {p, 4+p} in global rank order (the own
    # block is overwritten with identical bytes, keeping SPMD symmetric).
    ocat_sb = pool.tile([P, EC, L], BF16, tag="xbuf")
    cc_in = [
        dram.tile([P, 512], BF16, tag=f"ccin{i}", bufs=1, name=f"ccin{i}")
        for i in range(NPAIR * LT)
    ]
    cc_out = [
        dram.tile([2, P, 512], BF16, tag=f"ccout{i}", bufs=1, name=f"ccout{i}")
        for i in range(NPAIR * LT)
    ]

    pending_norm = [None]

    for p in range(NPAIR):
        for lq in range(LT):
            ot0 = psum.tile([P, 512], F32, tag="ot", bufs=2)
            ot1 = psum.tile([P, 512], F32, tag="ot", bufs=2)
            ots = (ot0, ot1)
            Lq = slice(lq * 512, (lq + 1) * 512)

            sc_ps = {}
            at_sb = {}

            def scores(lk):
                ps = psum.tile([P, 1024], F32, tag="sc", bufs=3)
                sc_ps[lk] = ps
                klt, koff = lk // 4, 512 + (lk % 4) * P
                for hh in range(2):
                    nc.tensor.matmul(
                        ps[:, hh * 512 : (hh + 1) * 512],
                        lhsT=qk_sb[hh * 64 : (hh + 1) * 64, p, klt, koff : koff + P],
                        rhs=qk_sb[hh * 64 : (hh + 1) * 64, p, lq, 0:512],
                        start=True,
                        stop=True,
                    )
                ps = sc_ps[lk]
                at = work.tile([P, 1024], BF16, tag="at", bufs=4)
                if lk in DVE_EXP_LKS:
                    nc.vector.tensor_scalar(
                        out=at[:].bitcast(I16), in0=ps[:], scalar1=sch_a,
                        scalar2=sch_b,
                        op0=mybir.AluOpType.mult, op1=mybir.AluOpType.add,
                    )
                else:
                    nc.scalar.activation(at[:], ps[:], Exp, scale=scale)
                at_sb[lk] = at[:]

            def av(lk):
                at = at_sb.pop(lk)
                sc_ps.pop(lk)
                for hh in range(2):
                    nc.tensor.matmul(
                        ots[hh][0:65, :],
                        lhsT=vaug_sb[:, lk, 2 * p + hh, :],
                        rhs=at[:, hh * 512 : (hh + 1) * 512],
                        start=(lk == 0),
                        stop=(lk == LKC - 1),
                    )

            scores(0)
            scores(1)
            scores(2)
            if pending_norm[0] is not None:  # prior iteration's norm tail
                pending_norm[0]()
                pending_norm[0] = None
            for lk in range(3, LKC):
                av(lk - 3)
                scores(lk)
            av(LKC - 3)
            av(LKC - 2)
            av(LKC - 1)

            # normalization: denominator in psum row 64 -> reciprocal ->
            # broadcast -> per-head multiply into stacked oloc layout
            # free the ot psum bufs ASAP: copy O_u + denominator row to
            # SBUF (head0 -> cols 0:512, head1 -> 512:1024).  The rest of
            # the normalization chain is deferred into the next iteration
            # (after its first scores) so it never delays boundary exps.
            ou = work.tile([65, 1024], F32, tag="ou", bufs=2)
            nc.vector.tensor_copy(out=ou[:, 0:512], in_=ot0[0:65, :])
            nc.vector.tensor_copy(out=ou[:, 512:1024], in_=ot1[0:65, :])

            def norm_tail(p=p, lq=lq, Lq=Lq, ou=ou):
                den0 = work.tile([1, 1024], F32, tag="den0", bufs=2)
                nc.sync.dma_start(den0[0:1, :], ou[64:65, :])
                rec = work.tile([1, 1024], F32, tag="rec", bufs=2)
                nc.vector.reciprocal_approx_fast(rec[0:1, :], den0[0:1, :])
                rbc = work.tile([64, 1024], F32, tag="rbc", bufs=2)
                nc.gpsimd.partition_broadcast(rbc[:], rec[0:1, :])
                # head 2p -> ocat partitions 0-63; head 2p+1 via tmp + DMA
                nc.gpsimd.tensor_mul(ocat_sb[0:64, p, Lq], ou[0:64, 0:512], rbc[:, 0:512])
                onrm = work.tile([64, 512], BF16, tag="onrm", bufs=2)
                nc.gpsimd.tensor_mul(onrm[:], ou[0:64, 512:1024], rbc[:, 512:1024])
                nc.sync.dma_start(ocat_sb[64:128, p, Lq], onrm[:])
                # stage + AllGather this (pair, lq) slice; unpack both
                # ranks into global feature order (chunk 4*rank + p)
                i = p * LT + lq
                nc.sync.dma_start(cc_in[i][:], ocat_sb[:, p, Lq])
                if use_collective:
                    nc.gpsimd.collective_compute(
                        "AllGather",
                        mybir.AluOpType.bypass,
                        replica_groups=[[2 * j, 2 * j + 1] for j in range(N_CORES // 2)],
                        ins=[cc_in[i][:].opt()],
                        outs=[cc_out[i][:].opt()],
                    )
                else:
                    nc.sync.dma_start(cc_out[i][0], cc_in[i][:])
                    nc.sync.dma_start(cc_out[i][1], cc_in[i][:])
                for r in range(2):
                    nc.sync.dma_start(ocat_sb[:, 4 * r + p, Lq], cc_out[i][r])

            pending_norm[0] = norm_tail

    pending_norm[0]()
    pending_norm[0] = None

    if taps is not None:
        nc.sync.dma_start(taps["qk"].ap(), qk_sb[:])
        nc.sync.dma_start(taps["vaug"].ap(), vaug_sb[:])
        nc.sync.dma_start(
            taps["ocat"].ap().rearrange("(c p) l -> p c l", p=P), ocat_sb[:]
        )

    # ---- output projection: y = Ocat^T.T @ wout_half, [L, E/2] ----
    proj_order = [0, 1, 2, 4, 5, 6, 3, 7]
    for lq in range(L // P):
        ps = psum.tile([P, 512], F32, tag="ot", bufs=2)
        for j, e in enumerate(proj_order):
            nc.tensor.matmul(
                ps[:],
                lhsT=ocat_sb[:, e, lq * P : (lq + 1) * P],
                rhs=wout_sb[:, e, :],
                start=(j == 0),
                stop=(j == EC - 1),
            )
        yt = work.tile([P, EH], F32, tag="yt", bufs=2)
        nc.scalar.copy(out=yt[:], in_=ps[:])
        nc.sync.dma_start(y.ap()[lq * P : (lq + 1) * P, :], yt[:])

    ctx.close()


@functools.lru_cache(maxsize=2)
def build_module(L=L_FULL, E=E_FULL, HC=H_FULL // 2, D=64, asserts=False,
                 debug_taps=False, use_collective=True):
    nc = bacc.Bacc(
        "TRN2",
        target_bir_lowering=False,
        debug=False,
        enable_asserts=asserts,
        num_devices=N_CORES,
    )
    A = HC * D
    xT = nc.dram_tensor("xT", [E, L], BF16, kind="ExternalInput")
    wqkv = nc.dram_tensor("wqkv", [E, 3 * A], BF16, kind="ExternalInput")
    wout = nc.dram_tensor("wout", [E, E // 2], BF16, kind="ExternalInput")
    cosT = nc.dram_tensor("cosT", [128, 2 * L], BF16, kind="ExternalInput")
    sinT = nc.dram_tensor("sinT", [128, 2 * L], BF16, kind="ExternalInput")
    y = nc.dram_tensor("y", [L, E // 2], F32, kind="ExternalOutput")
    taps = None
    if debug_taps:
        taps = {
            "qk": nc.dram_tensor("qk_dbg", [128, HC // 2, L // 512, 1024], BF16, kind="ExternalOutput"),
            "vaug": nc.dram_tensor(
                "vaug_dbg", [128, L // 128, HC, D + 1], BF16, kind="ExternalOutput"
            ),
            "ocat": nc.dram_tensor("ocat_dbg", [E, L], BF16, kind="ExternalOutput"),
        }
    with tile.TileContext(nc) as tc:
        _emit(tc, nc, xT, wqkv, wout, cosT, sinT, y, L, E, HC, D, taps=taps,
              use_collective=use_collective)
    nc.compile()
    return nc


def _rope_tables(L, D):
    """cos/sin tables in the de-interleaved 32-row layout, stacked x4.

    Row p (p in [0,32)): frequency p (covers original dims 2p / 2p+1).
    sin is pre-signed for the post-swap add: blocks [+s, -s, +s, -s].
    """
    half = D // 2
    inv_freq = 1.0 / (ROPE_THETA ** (np.arange(0, D, 2, dtype=np.float64) / D))
    freqs = np.arange(L, dtype=np.float64)[None, :] * inv_freq[:, None]  # [32, L]
    cos32 = np.cos(freqs)
    sin32 = np.sin(freqs)
    bf = ml_dtypes.bfloat16
    cos = np.tile(cos32, (128 // half, 1))
    sin_block = np.concatenate([sin32, -sin32], axis=0)  # [64, L]
    sin = np.tile(sin_block, (2, 1))

    def dup_qk(tab):
        # [128, L] -> [128, LT, 1024]: per 512-tile, duplicate for q|k
        t = tab.reshape(128, L // 512, 512)
        return np.concatenate([t, t], axis=2).reshape(128, 2 * L).astype(bf)

    return dup_qk(cos), dup_qk(sin)


def _deint_cols(base, h, D):
    """Column indices of head h (offset base), even dims then odd dims."""
    cols = base + h * D + np.arange(D)
    return np.concatenate([cols[0::2], cols[1::2]])


def make_core_inputs(x, w_qkv, w_out, H=H_FULL, D=64):
    """Per-core input dicts from the full (unsharded) fp32 inputs."""
    Bv, L, E = x.shape
    HC = H // (N_CORES // Bv)
    A_full = H * D
    bf = ml_dtypes.bfloat16
    cos, sin = _rope_tables(L, D)
    in_maps = []
    for c in range(N_CORES):
        b, g = c // 2, c % 2
        xT = np.ascontiguousarray(x[b].T).astype(bf)
        qcols = []
        kcols = []
        vcols = []
        for p in range(HC // 2):
            for hh in range(2):
                h = g * HC + 2 * p + hh
                qcols.append(_deint_cols(0, h, D))
                kcols.append(_deint_cols(A_full, h, D))
        for hl in range(HC):
            h = g * HC + hl
            vcols.append(2 * A_full + h * D + np.arange(D))
        cols = np.concatenate(qcols + kcols + vcols)
        wqkv_c = np.ascontiguousarray(w_qkv[:, cols]).astype(bf)
        wout_c = np.ascontiguousarray(
            w_out[:, g * (E // 2) : (g + 1) * (E // 2)]
        ).astype(bf)
        in_maps.append(
            {
                "xT": xT,
                "wqkv": wqkv_c,
                "wout": wout_c,
                "cosT": cos,
                "sinT": sin,
            }
        )
    return in_maps


def kernel(x, w_qkv, w_out):
    x = np.asarray(x)
    w_qkv = np.asarray(w_qkv)
    w_out = np.asarray(w_out)
    Bv, L, E = x.shape
    nc = build_module(L=L, E=E)
    in_maps = make_core_inputs(x, w_qkv, w_out)
    res = run_bass_kernel_spmd(nc, in_maps, core_ids=list(range(N_CORES)))
    out = np.empty((Bv, L, E), dtype=np.float32)
    EH = E // 2
    for b in range(Bv):
        out[b][:, :EH] = res.results[2 * b]["y"]
        out[b][:, EH:] = res.results[2 * b + 1]["y"]
    return out
